# revision 5
# baseline (speedup 1.0000x reference)
"""Channel-attention (SE) layer for Trainium2, data-parallel over batch on 8 NeuronCores.

Reference computation (per sample):
    avg = mean(x, HW); mx = max(x, HW)                    # [C]
    attn = sigmoid(mlp(avg) + mlp(mx))                    # mlp: relu(v@w1+b1)@w2+b2
    out = x * attn

Per-core strategy (4 samples of [3136, 256] fp32 each):
  - Stream each sample in as [128, 24, 256] (+ a [64, 256] leftover), position-partition
    layout, one big HWDGE DMA per sample. Sample stays resident in SBUF between pooling
    and the final multiply, so HBM traffic is read-once + write-once (~25.7 MB/core).
  - Sum-pool on the TensorE: ones[128,1] stationary, 25 accumulating matmuls -> [1, C].
  - Max-pool on VectorE: one strided tensor_reduce over chunks -> [128, C], leftover
    fixup, then PE-transpose + reduce to get channel-partition maxima [128,1] x2.
  - Tiny shared-MLP entirely on-chip in channel-partition layout; bias b2 folded into an
    augmented 33-row w2 matmul (each path gets +b2, matching the reference exactly).
  - attn row broadcast to 128 partitions with a K=1 outer-product matmul, then one
    broadcast tensor_mul over the resident sample and HWDGE store via GpSimd's queue.
"""

import numpy as np

B, H, W, C = 32, 56, 56, 256
R = 8
CR = C // R  # 32
N_CORES = 8
SPB = B // N_CORES  # 4 samples per core
HW = H * W  # 3136
NCH = HW // 128  # 24 full 128-row chunks per sample
REM = HW - NCH * 128  # 64 leftover rows
ROWS = SPB * HW  # 12544 rows per core

_cache = {}
LAST_RESULT = None  # BassKernelResults of the most recent run (for profiling)


def _build_nc(repeat: int = 1):
    """Build the per-core bass program. `repeat` unrolls the whole body N times
    (identical work each iteration) — used only for self-timing on hardware,
    since NTFF profiling is unavailable over this axon tunnel."""
    import concourse.bacc as bacc
    import concourse.mybir as mybir
    import concourse.tile as tile
    from concourse.masks import make_identity

    f32 = mybir.dt.float32
    AF = mybir.ActivationFunctionType
    AX = mybir.AxisListType

    nc = bacc.Bacc("TRN2", target_bir_lowering=False, debug=False)

    x = nc.dram_tensor("x", [ROWS, C], f32, kind="ExternalInput").ap()
    w1 = nc.dram_tensor("w1", [C, CR], f32, kind="ExternalInput").ap()
    b1c = nc.dram_tensor("b1c", [CR, 1], f32, kind="ExternalInput").ap()
    w2b = nc.dram_tensor("w2b", [CR + 1, C], f32, kind="ExternalInput").ap()
    out = nc.dram_tensor("out", [ROWS, C], f32, kind="ExternalOutput").ap()

    with tile.TileContext(nc) as tc:
        with (
            tc.tile_pool(name="const", bufs=1) as constp,
            tc.tile_pool(name="xbig", bufs=4) as xbig,
            tc.tile_pool(name="work", bufs=4) as work,
            tc.tile_pool(name="ps_acc", bufs=2, space="PSUM") as ps_acc,
            tc.tile_pool(name="ps_tiny", bufs=4, space="PSUM") as ps_tiny,
            tc.tile_pool(name="ps_big", bufs=2, space="PSUM") as ps_big,
        ):
            # ---- constants ----
            w1_sb = constp.tile([128, 2, CR], f32)
            nc.sync.dma_start(w1_sb, w1.rearrange("(t p) m -> p t m", p=128))
            b1_sb = constp.tile([CR, 1], f32)
            nc.sync.dma_start(b1_sb, b1c)
            w2b_sb = constp.tile([CR + 1, C], f32)
            nc.sync.dma_start(w2b_sb, w2b)
            ones_col = constp.tile([128, 1], f32)
            nc.gpsimd.memset(ones_col, 1.0)
            ones_row = constp.tile([1, 128], f32)
            nc.gpsimd.memset(ones_row, 1.0)
            ident = constp.tile([128, 128], f32)
            make_identity(nc, ident)

            for s in range(SPB * repeat):
                s = s % SPB
                base = s * HW

                # ---- load sample (position-partition layout) ----
                xs = xbig.tile([128, NCH, C], f32, tag="xs")
                xr = work.tile([128, C], f32, tag="xr")
                nc.sync.dma_start(
                    xs, x[base : base + NCH * 128, :].rearrange("(n p) c -> p n c", p=128)
                )
                # zero-pad leftover rows so the sum matmul can use full K=128
                nc.gpsimd.memset(xr[REM:, :], 0.0)
                nc.sync.dma_start(xr[:REM, :], x[base + NCH * 128 : base + HW, :])

                # ---- sum pooling on PE: acc[0, c] = sum over positions ----
                acc = ps_acc.tile([1, C], f32, tag="acc")
                for n in range(NCH):
                    nc.tensor.matmul(
                        acc, ones_col, xs[:, n, :], start=(n == 0), stop=False
                    )
                nc.tensor.matmul(acc, ones_col, xr, start=False, stop=True)

                # ---- max pooling on DVE ----
                m1 = work.tile([128, C], f32, tag="m1")
                nc.vector.reduce_max(m1, xs.transpose([0, 2, 1]), axis=AX.X)
                nc.vector.tensor_max(m1[:REM, :], m1[:REM, :], xr[:REM, :])

                # transpose [128, C] max image to channel-partition, reduce over positions
                mt = ps_big.tile([128, 2, 128], f32, tag="big")
                nc.tensor.transpose(mt[:, 0, :], m1[:, 0:128], ident)
                nc.tensor.transpose(mt[:, 1, :], m1[:, 128:256], ident)

                # vt[:, t, 0] = avg (channels t*128..), vt[:, t, 1] = max
                vt = work.tile([128, 2, 2], f32, tag="vt")
                nc.vector.reduce_max(vt[:, 0, 1:2], mt[:, 0, :], axis=AX.X)
                nc.vector.reduce_max(vt[:, 1, 1:2], mt[:, 1, :], axis=AX.X)

                # ---- mean row, transposed to channel-partition via K=1 matmul ----
                mean_row = work.tile([1, C], f32, tag="mean_row")
                nc.scalar.activation(mean_row, acc, AF.Copy, scale=1.0 / HW)
                at = ps_tiny.tile([128, 2], f32, tag="tiny")
                nc.tensor.matmul(
                    at[:, 0:1], mean_row[0:1, 0:128], ones_row[0:1, 0:1],
                    start=True, stop=True,
                )
                nc.tensor.matmul(
                    at[:, 1:2], mean_row[0:1, 128:256], ones_row[0:1, 0:1],
                    start=True, stop=True,
                )
                nc.scalar.copy(vt[:, 0, 0:1], at[:, 0:1])
                nc.scalar.copy(vt[:, 1, 0:1], at[:, 1:2])

                # ---- shared MLP, both pooled vectors at once ----
                # hp = w1.T @ V.T  -> [CR, 2] (col 0 avg-path, col 1 max-path)
                hp = ps_tiny.tile([CR, 2], f32, tag="tiny")
                nc.tensor.matmul(hp, w1_sb[:, 0, :], vt[:, 0, :], start=True, stop=False)
                nc.tensor.matmul(hp, w1_sb[:, 1, :], vt[:, 1, :], start=False, stop=True)

                hs = work.tile([CR + 1, 2], f32, tag="hs")
                nc.scalar.activation(hs[0:CR, :], hp, AF.Relu, bias=b1_sb, scale=1.0)
                nc.gpsimd.memset(hs[CR : CR + 1, :], 1.0)

                # z = mlp(avg) + mlp(max), accumulated in PSUM: each matmul via the
                # augmented [w2; b2] adds one +b2, so the sum carries 2*b2 as required.
                apz = ps_tiny.tile([1, C], f32, tag="tiny")
                nc.tensor.matmul(apz, hs[:, 0:1], w2b_sb, start=True, stop=False)
                nc.tensor.matmul(apz, hs[:, 1:2], w2b_sb, start=False, stop=True)
                arow = work.tile([1, C], f32, tag="arow")
                nc.scalar.activation(arow, apz, AF.Sigmoid)

                # broadcast attn row to all 128 partitions (K=1 outer product)
                bc = ps_big.tile([128, C], f32, tag="big")
                nc.tensor.matmul(bc, ones_row, arow, start=True, stop=True)
                attn = work.tile([128, C], f32, tag="attn")
                nc.scalar.copy(attn, bc)

                # ---- multiply in place + store ----
                nc.vector.tensor_mul(
                    xs, xs, attn.unsqueeze(1).to_broadcast([128, NCH, C])
                )
                nc.vector.tensor_mul(xr[:REM, :], xr[:REM, :], attn[:REM, :])
                nc.gpsimd.dma_start(
                    out[base : base + NCH * 128, :].rearrange("(n p) c -> p n c", p=128),
                    xs,
                )
                nc.gpsimd.dma_start(out[base + NCH * 128 : base + HW, :], xr[:REM, :])

    nc.compile()
    return nc


def _get_nc():
    if "nc" not in _cache:
        _cache["nc"] = _build_nc()
    return _cache["nc"]


def _shard_inputs(x, w1, b1, w2, b2):
    shards = np.ascontiguousarray(x.reshape(N_CORES, ROWS, C))
    w2b = np.ascontiguousarray(
        np.concatenate([w2, b2[None, :]], axis=0).astype(np.float32)
    )
    b1c = np.ascontiguousarray(b1[:, None].astype(np.float32))
    return [
        {"x": shards[i], "w1": w1, "b1c": b1c, "w2b": w2b} for i in range(N_CORES)
    ]


def kernel(**inputs) -> np.ndarray:
    global LAST_RESULT
    x = np.asarray(inputs["x"], dtype=np.float32)
    w1 = np.asarray(inputs["w1"], dtype=np.float32)
    b1 = np.asarray(inputs["b1"], dtype=np.float32)
    w2 = np.asarray(inputs["w2"], dtype=np.float32)
    b2 = np.asarray(inputs["b2"], dtype=np.float32)

    from concourse.bass_utils import run_bass_kernel_spmd

    nc = _get_nc()
    in_maps = _shard_inputs(x, w1, b1, w2, b2)
    res = run_bass_kernel_spmd(nc, in_maps, core_ids=list(range(N_CORES)))
    LAST_RESULT = res
    outs = np.stack([res.results[i]["out"] for i in range(N_CORES)], axis=0)
    return outs.reshape(B, H, W, C)


# revision 33
# speedup vs baseline: 1.0765x; 1.0765x over previous
"""Channel-attention (SE) layer for Trainium2, data-parallel over batch on 8 NeuronCores.

Reference computation (per sample):
    avg = mean(x, HW); mx = max(x, HW)                    # [C]
    attn = sigmoid(mlp(avg) + mlp(mx))                    # mlp: relu(v@w1+b1)@w2+b2
    out = x * attn

Per-core strategy (4 samples of [3136, 256] fp32 each):
  - Stream each sample in as [128, 24, 256] (+ a [64, 256] leftover), position-partition
    layout, one big HWDGE DMA per sample. The sample stays resident in SBUF between
    pooling and the final multiply, so HBM traffic is read-once + write-once
    (~25.7 MB/core ≈ 72 us at ~358 GB/s — the roofline for this memory-bound op).
  - Sum-pool on TensorE: ones[128,1] stationary, N=512 accumulating fp32r matmuls
    (fp32r streams 1 col/cycle vs 4 for fp32; pooling precision is ample).
  - Max-pool on VectorE: strided tensor_reduce over chunks -> [128, C], leftover fixup,
    then PE-transpose + reduce to land channel-partition maxima directly in the MLP
    input tile.
  - Mean transposed to channel-partition with K=1 outer-product matmuls against a
    [1,1] tile holding 1/HW (scale + transpose + even/odd-half add all fused into the
    PSUM accumulation).
  - Tiny shared-MLP on-chip; b2 folded into an augmented 33-row [w2; b2] matmul whose
    two accumulating passes contribute the required 2*b2.
  - attn row broadcast to 128 partitions by a K=1 outer product into PSUM; the final
    multiply reads it straight from PSUM (broadcast along the chunk dim), writes xs in
    place, and stores via the ScalarE HWDGE queue (SP queue carries the loads).
  - The per-sample body is emitted in software-pipelined phase order so each engine's
    in-order stream never interleaves a long wait between samples.
"""

import numpy as np

B, H, W, C = 32, 56, 56, 256
R = 8
CR = C // R  # 32
N_CORES = 8
SPB = B // N_CORES  # 4 samples per core
HW = H * W  # 3136
NCH = HW // 128  # 24 full 128-row chunks per sample
REM = HW - NCH * 128  # 64 leftover rows
ROWS = SPB * HW  # 12544 rows per core

_cache = {}
LAST_RESULT = None  # BassKernelResults of the most recent run (for profiling)


def _build_nc(repeat: int = 1):
    """Build the per-core bass program. `repeat` unrolls the whole body N times
    (identical work each iteration) — used only for self-timing on hardware,
    since NTFF profiling is unavailable over this axon tunnel."""
    import concourse.bacc as bacc
    import concourse.mybir as mybir
    import concourse.tile as tile
    from concourse import bass_isa
    from concourse.tile import add_dep_helper

    f32 = mybir.dt.float32
    f32r = mybir.dt.float32r
    AF = mybir.ActivationFunctionType
    AX = mybir.AxisListType
    ALU = mybir.AluOpType

    nc = bacc.Bacc("TRN2", target_bir_lowering=False, debug=False)

    x = nc.dram_tensor("x", [ROWS, C], f32, kind="ExternalInput").ap()
    w1 = nc.dram_tensor("w1", [C, CR], f32, kind="ExternalInput").ap()
    b1c = nc.dram_tensor("b1c", [CR, 1], f32, kind="ExternalInput").ap()
    w2b = nc.dram_tensor("w2b", [CR + 1, C], f32, kind="ExternalInput").ap()
    out = nc.dram_tensor("out", [ROWS, C], f32, kind="ExternalOutput").ap()

    with tile.TileContext(nc) as tc:
        NH = NCH // 2  # chunks per half-load
        with (
            tc.tile_pool(name="const", bufs=1) as constp,
            tc.tile_pool(name="xbig", bufs=4) as xbig,
            tc.tile_pool(name="work", bufs=4) as work,
            tc.tile_pool(name="ps_acc", bufs=2, space="PSUM") as ps_acc,
            tc.tile_pool(name="ps_tiny", bufs=3, space="PSUM") as ps_tiny,
            tc.tile_pool(name="ps_big", bufs=3, space="PSUM") as ps_big,
        ):
            # ---- constants (on-device generated ones first; DMA'd consts are
            # emitted after sample 0's loads so they don't delay the pipeline) ----
            w1_sb = constp.tile([128, 2, CR], f32)
            b1_sb = constp.tile([CR, 1], f32)
            w2b_sb = constp.tile([CR + 1, C], f32)
            ones_col = constp.tile([128, 1], f32)
            nc.gpsimd.memset(ones_col, 1.0)
            half_col = constp.tile([128, 1], f32)
            nc.gpsimd.memset(half_col, 0.5)
            ones_row = constp.tile([1, 128], f32)
            nc.gpsimd.memset(ones_row, 1.0)
            invhw = constp.tile([1, 1], f32)
            nc.gpsimd.memset(invhw, 1.0 / HW)
            warm_sb = constp.tile([128, 2 * C], f32)
            nc.gpsimd.memset(warm_sb, 0.0)

            first_iter = True
            for _ in range(repeat):
                xs, bc = [], []

                # ---- loads first (SP HWDGE queue), split in halves for finer
                # load/compute/store interleaving. The 64 leftover rows are loaded
                # TWICE into chunk NCH (rows 0:64 and 64:128): the max is unaffected
                # by duplicates and the sum matmul weights that chunk by 0.5, so
                # both pools fold the leftover in with no extra fixup ops. ----
                last_load = [None]

                def load_sample(s):
                    base = s * HW
                    # +2 chunk slots: NCH = duplicated leftover rows, NCH+1 =
                    # scratch for the first-half max image (folded into red_b)
                    xs.append(
                        xbig.tile([128, NCH + 2, C], f32, tag="xs", name=f"xs{s}")
                    )
                    for h in range(2):
                        li = nc.sync.dma_start(
                            xs[s][:, h * NH : (h + 1) * NH, :],
                            x[
                                base + h * NH * 128 : base + (h + 1) * NH * 128, :
                            ].rearrange("(n p) c -> p n c", p=128),
                        )
                    lx = x[base + NCH * 128 : base + HW, :]
                    nc.sync.dma_start(xs[s][:REM, NCH, :], lx)
                    nc.sync.dma_start(xs[s][REM:, NCH, :], lx)
                    # remember the last BIG load: stores are gated on it so they
                    # never steal DMA bandwidth from a load that gates compute
                    last_load[0] = li.ins

                load_sample(0)
                if first_iter:
                    nc.sync.dma_start(w1_sb, w1.rearrange("(t p) m -> p t m", p=128))
                    nc.sync.dma_start(b1_sb, b1c)
                    nc.sync.dma_start(w2b_sb, w2b)
                    # warm the PE HAM clock gate with tiny matmuls while the first
                    # loads stream in, so the real sums run at 2.4 GHz from the start
                    wacc = ps_acc.tile([1, 2 * C], f32, tag="acc", name="wacc")
                    for i in range(64):
                        nc.tensor.matmul(
                            wacc[:, 0:16],
                            ones_col,
                            warm_sb[:, 0:16],
                            start=(i == 0),
                            stop=(i == 63),
                        )
                    first_iter = False
                for s in range(1, SPB):
                    load_sample(s)

                def pool_and_mlp(s):
                    # ---- sum pooling on PE (fp32, N=512): acc[0, 0:C] even
                    # chunks, acc[0, C:2C] odd chunks; halves re-added for free
                    # by the transpose matmuls below ----
                    acc = ps_acc.tile([1, 2 * C], f32, tag="acc", name=f"acc{s}")
                    for n in range(0, NCH, 2):
                        nc.tensor.matmul(
                            acc,
                            ones_col,
                            xs[s][:, n : n + 2, :].rearrange("p n c -> p (n c)"),
                            start=(n == 0),
                            stop=False,
                        )
                    # leftover chunk holds its rows twice -> weight by 0.5
                    nc.tensor.matmul(
                        acc[:, 0:C],
                        half_col,
                        xs[s][:, NCH, :],
                        start=False,
                        stop=True,
                    )

                    # ---- max pooling on DVE (split per half-load; duplicated
                    # leftover rows make chunk NCH safe to include). The first
                    # reduce lands in xs chunk slot NCH+1 so the second reduce
                    # folds it in for free -> mx is the full per-row max. ----
                    mx = work.tile([128, C], f32, tag="mx", name=f"mx{s}")
                    nc.vector.reduce_max(
                        xs[s][:, NCH + 1, :],
                        xs[s][:, 0:NH, :].transpose([0, 2, 1]),
                        axis=AX.X,
                    )
                    nc.vector.reduce_max(
                        mx, xs[s][:, NH : NCH + 2, :].transpose([0, 2, 1]), axis=AX.X
                    )

                    # ---- latency-critical pooled-vector finishers + MLP. Runs
                    # entirely on GpSimd/PE/ACT (never queues on the DVE stream)
                    # and at high priority so mul_{s} is never left waiting. ----
                    with tc.high_priority():
                        # cross-partition max on GpSimd; row 0 holds max[C]
                        mxr = work.tile([128, C], f32, tag="mxr", name=f"mxr{s}")
                        nc.gpsimd.partition_all_reduce(
                            mxr, mx, channels=128, reduce_op=bass_isa.ReduceOp.max
                        )

                        # mean: PSUM -> SBUF, then channel-partition transpose of
                        # both pooled rows via K=1 outer products (for the mean,
                        # the 1/HW scale and the even/odd-half add are fused in)
                        sum_sb = work.tile(
                            [1, 2 * C], f32, tag="sum_sb", name=f"sums{s}"
                        )
                        nc.scalar.copy(sum_sb, acc)
                        # at cols: 0,1 = avg lo/hi half; 2,3 = max lo/hi half
                        at = ps_tiny.tile([128, 4], f32, tag="tiny", name=f"at{s}")
                        for h in range(2):
                            nc.tensor.matmul(
                                at[:, h : h + 1],
                                sum_sb[0:1, 128 * h : 128 * (h + 1)],
                                invhw,
                                start=True,
                                stop=False,
                            )
                            nc.tensor.matmul(
                                at[:, h : h + 1],
                                sum_sb[0:1, C + 128 * h : C + 128 * (h + 1)],
                                invhw,
                                start=False,
                                stop=True,
                            )
                            nc.tensor.matmul(
                                at[:, 2 + h : 3 + h],
                                mxr[0:1, 128 * h : 128 * (h + 1)],
                                ones_row[0:1, 0:1],
                                start=True,
                                stop=True,
                            )
                        # vt[:, t, 0] = avg, vt[:, t, 1] = max (channel half t)
                        vt = work.tile([128, 2, 2], f32, tag="vt", name=f"vt{s}")
                        nc.scalar.copy(vt[:, 0, 0:1], at[:, 0:1])
                        nc.scalar.copy(vt[:, 1, 0:1], at[:, 1:2])
                        nc.scalar.copy(vt[:, 0, 1:2], at[:, 2:3])
                        nc.scalar.copy(vt[:, 1, 1:2], at[:, 3:4])

                        # shared MLP:
                        # hp = w1.T @ V.T -> [CR, 2] (col 0 avg-path, col 1 max)
                        hp = ps_tiny.tile([CR, 2], f32, tag="tiny", name=f"hp{s}")
                        nc.tensor.matmul(
                            hp, w1_sb[:, 0, :], vt[:, 0, :], start=True, stop=False
                        )
                        nc.tensor.matmul(
                            hp, w1_sb[:, 1, :], vt[:, 1, :], start=False, stop=True
                        )
                        # relu(hp + b1): ACT copies PSUM->SBUF (table-free), GpSimd
                        # applies add+max (keeps ScalarE's LUT pinned to Sigmoid
                        # and the chain off the DVE queue)
                        hs = work.tile([CR + 1, 2], f32, tag="hs", name=f"hs{s}")
                        nc.vector.tensor_scalar(
                            hs[0:CR, :],
                            hp,
                            scalar1=b1_sb,
                            scalar2=0.0,
                            op0=ALU.add,
                            op1=ALU.max,
                        )
                        nc.gpsimd.memset(hs[CR : CR + 1, :], 1.0)

                        # z = mlp(avg) + mlp(max) in PSUM; each matmul via the
                        # augmented [w2; b2] adds one +b2 -> the sum carries 2*b2.
                        apz = ps_tiny.tile([1, C], f32, tag="tiny", name=f"apz{s}")
                        nc.tensor.matmul(
                            apz, hs[:, 0:1], w2b_sb, start=True, stop=False
                        )
                        nc.tensor.matmul(
                            apz, hs[:, 1:2], w2b_sb, start=False, stop=True
                        )
                        arow = work.tile([1, C], f32, tag="arow", name=f"arow{s}")
                        nc.scalar.activation(arow, apz, AF.Sigmoid)

                        # broadcast attn row to all 128 partitions; the multiply
                        # reads it straight from PSUM
                        bc.append(ps_big.tile([128, C], f32, tag="big", name=f"bc{s}"))
                        nc.tensor.matmul(bc[s], ones_row, arow, start=True, stop=True)

                def mul_and_store(s):
                    # multiply in place + store (ScalarE HWDGE queue), per half;
                    # the second half includes the leftover chunk (its duplicate
                    # rows are multiplied too but never stored)
                    base = s * HW
                    nc.vector.tensor_mul(
                        xs[s][:, 0:NH, :],
                        xs[s][:, 0:NH, :],
                        bc[s].unsqueeze(1).to_broadcast([128, NH, C]),
                    )
                    nc.sync.dma_start(
                        out[base : base + NH * 128, :].rearrange(
                            "(n p) c -> p n c", p=128
                        ),
                        xs[s][:, 0:NH, :],
                    )
                    nc.vector.tensor_mul(
                        xs[s][:, NH : NCH + 1, :],
                        xs[s][:, NH : NCH + 1, :],
                        bc[s].unsqueeze(1).to_broadcast([128, NH + 1, C]),
                    )
                    nc.sync.dma_start(
                        out[base + NH * 128 : base + NCH * 128, :].rearrange(
                            "(n p) c -> p n c", p=128
                        ),
                        xs[s][:, NH:NCH, :],
                    )
                    nc.sync.dma_start(
                        out[base + NCH * 128 : base + HW, :], xs[s][:REM, NCH, :]
                    )

                # Skewed software pipeline: sample s-1's multiply+store is emitted
                # between per-sample blocks so the store DMAs interleave with the
                # remaining loads and the DVE stream never waits on an MLP round
                # trip it just issued.
                for s in range(SPB):
                    pool_and_mlp(s)
                    if s >= 1:
                        mul_and_store(s - 1)
                mul_and_store(SPB - 1)

    nc.compile()
    return nc


def _get_nc():
    if "nc" not in _cache:
        _cache["nc"] = _build_nc()
    return _cache["nc"]


def _shard_inputs(x, w1, b1, w2, b2):
    shards = np.ascontiguousarray(x.reshape(N_CORES, ROWS, C))
    w2b = np.ascontiguousarray(
        np.concatenate([w2, b2[None, :]], axis=0).astype(np.float32)
    )
    b1c = np.ascontiguousarray(b1[:, None].astype(np.float32))
    return [
        {"x": shards[i], "w1": w1, "b1c": b1c, "w2b": w2b} for i in range(N_CORES)
    ]


def kernel(**inputs) -> np.ndarray:
    global LAST_RESULT
    x = np.asarray(inputs["x"], dtype=np.float32)
    w1 = np.asarray(inputs["w1"], dtype=np.float32)
    b1 = np.asarray(inputs["b1"], dtype=np.float32)
    w2 = np.asarray(inputs["w2"], dtype=np.float32)
    b2 = np.asarray(inputs["b2"], dtype=np.float32)

    from concourse.bass_utils import run_bass_kernel_spmd

    nc = _get_nc()
    in_maps = _shard_inputs(x, w1, b1, w2, b2)
    res = run_bass_kernel_spmd(nc, in_maps, core_ids=list(range(N_CORES)))
    LAST_RESULT = res
    outs = np.stack([res.results[i]["out"] for i in range(N_CORES)], axis=0)
    return outs.reshape(B, H, W, C)
